# revision 34
# baseline (speedup 1.0000x reference)
"""Trainium2 Bass kernel for the SOCS lithography simulator.

Reference math (per batch b):
    aerial = sum_k s_k * | cIFFT2( cFFT2(mask_b) * pad_center(kernels[k]) ) |^2
    resist = sigmoid(50*(aerial - 0.225));  printed = (aerial > 0.225)

Band-limited formulation (see git history of this file for derivation):
    Mhat  = A @ x @ A.T          A = rows 494:529 of the centered DFT matrix
    G_k   = Mhat * (sqrt(s_k) * kernels[k])                 [35,35] cplx
    F_k   = C @ G_k @ C.T        C = inverse-DFT samples at 72 stride-14 pts
    aer_c = sum_k |F_k|^2        exact coarse samples of aerial
    aerial = U @ aer_c @ U.T     U = Re(E pinv(V)) [1024,72]

Optimizations beyond the 50.4us baseline (final: ~47.3us):
  * input DMA on ONE ring in strict priority order: atp_y (stage-1 cols),
    x in 8 chunks of [128,1024] (1 DRAM row per partition, 2KB runs),
    then atp99 / kri / cc / uc.  One dma_start's descriptors fan out over
    all 16 DMA engines, so a single ring = serial arrival: stage 1 starts
    at ~11us and pipelines chunk-by-chunk under the x DMA.
  * atp strictly before x: the PE streams atp_sb during stage 1, and
    concurrent DMA writes into the tile being streamed slow every matmul
    ~20% (59ns -> 71ns per 70-col matmul, measured).
  * stages 2a-2d run in 2 super-rounds of 6 kernel pairs: 2a half ->
    6 pair-matmuls into one [99,1024] psum tile -> 4 copies of 432 cols
    (2 scalar + 2 vector; psum-read copies cost ~0.4-0.6us nearly
    independent of size, so few big copies beat many small ones) ->
    2 x (2d group matmuls + scalar SQUARE + vector presum a[g]).
  * intensity folds exploit linearity: S = fold6(a0+a1+a2) pre-folds
    during the last square; after sq3 only fold6(a3) (3 ops) remains.
    Stage 5a accumulates zp = S@uht (early, hidden) + d3@uht (late).
  * stage-5b psum tiles bufs=3 so block k+2's matmuls don't wait on
    block k's copies; output copies/DMA split in 512-col halves.
Measured and rejected:
  * PE p-state: clock ramps 1.2->2.4GHz after ~3us continuous tensor busy,
    runs hot ~3us, then throttles back and does NOT re-ramp even under
    gap-free load.  Filler matmuls are useless (and the Tile scheduler
    hoists them to the front of the queue anyway).
  * gpsimd tensor_add: ~1us per 432 cols (eff 0.42) vs vector 380ns; only
    memsets and DMA issue belong there.
  * NCC_IBIR297: SBUF TensorTensor inputs must share a base partition -> the
    kri swapped copy cannot be replaced by cross-block operands.
  * NCC_IBVF027: an instruction may read only ONE input from PSUM -- even
    the same tile twice (no DVE square-from-psum; no psum+psum adds).
  * dma_start cannot touch PSUM (SBUF/DRAM only): psum evacuation is a
    fixed DVE/ACT tax (~12k cols/core), the mid-section bottleneck.
  * cross-engine semaphore latency ~0.7us/hop: the serial ladder
    (copies->matmul->square->fold->matmul->copy->DMA) pays it ~12 times.
  * 16 small w99 copies (216 cols) cost 6.9us vs 8 big ones (432) 3.4us.

Hardware rules learned (cost a debug cycle each, do not regress):
  * a start=True matmul clears has_written bits for its whole PSUM bank ->
    concurrent accumulation chains need one bank each; single-shot
    (start+stop) matmuls may share a bank.
  * matmul PSUM output regions must not cross a 2KB bank boundary.
  * engine AP partition offsets must be multiples of 32 (hence the
    0:35 / 64:99 "99-row stack" layout used everywhere).
  * GPSIMD cannot read PSUM; DVE/ACT can read at most one PSUM operand.
  * collective_compute has a ~10us floor per op on this fabric (first one
    ~40us) - pair-wise k/y-splits via collectives do not pay off here.

Sharding: 8 cores; core c handles batch c//2 and output row-half c%2.
Each core runs stages 1-4 for its batch and half of stage 5. No collectives.

Self-contained: shapes/constants hardcoded, no sibling imports.
"""

import os

import numpy as np

N = 1024
B, K, HK = 4, 24, 35
PT = (N - HK) // 2          # 494
NC = 72                     # coarse grid samples (stride 14; >= 69 needed)
NF = 2 * HK - 1             # 69 product frequencies
RESIST_THRESHOLD = 0.225
RESIST_STEEPNESS = 50.0


# ---------------------------------------------------------------- host matrices
def _host_matrices():
    u = np.arange(HK)[:, None]          # 0..34  (centered freq u-18)
    y = np.arange(N)[None, :]
    A = np.exp(-2j * np.pi * ((u + PT - N // 2) * (y - N // 2)) / N)  # [35,1024]
    ym = 14 * np.arange(NC)
    Cs = np.exp(2j * np.pi * ((np.arange(HK)[None, :] - 18)
                              * (ym[:, None] - 512)) / N) / N         # [72,35]
    f = np.arange(-(NF // 2), NF // 2 + 1)
    V = np.exp(2j * np.pi * (f[None, :] * (ym[:, None] - 512)) / N)   # [72,69]
    E = np.exp(2j * np.pi * (f[None, :]
                             * (np.arange(N)[:, None] - 512)) / N)    # [1024,69]
    U = np.ascontiguousarray((E @ np.linalg.pinv(V)).real)            # [1024,72]

    atp = np.empty((N, 2 * HK), np.float32)          # [1024, 70]  A^T packed
    atp[:, :HK] = A.real.T
    atp[:, HK:] = A.imag.T
    ctr = np.ascontiguousarray(Cs.real.T, np.float32)   # [35,72] Ctr[q,m]=ReC[m,q]
    cti = np.ascontiguousarray(Cs.imag.T, np.float32)
    # ctp99: stacked rhs for stage 2c (contract Re/Im of G in one matmul)
    ctp99 = np.zeros((99, 2 * NC), np.float32)
    ctp99[0:35] = np.concatenate([ctr, cti], axis=1)        # top: [ctr | cti]
    ctp99[64:99] = np.concatenate([-cti, ctr], axis=1)      # bot: [-cti | ctr]
    # cc99: stacked stationary for stage 2d. col block 0: Re out, 1: Im out
    cc99 = np.zeros((99, 2 * NC), np.float32)
    cc99[0:35, 0:NC] = ctr
    cc99[64:99, 0:NC] = -cti
    cc99[0:35, NC:2 * NC] = cti
    cc99[64:99, NC:2 * NC] = ctr
    ut = np.ascontiguousarray(U.T, np.float32)          # [72,1024]
    return atp, ctp99, cc99, ut, U.astype(np.float32)


# ---------------------------------------------------------------- bass program
def _build_program():
    import concourse.bass as bass
    import concourse.mybir as mybir
    import concourse.tile as tile
    from concourse import bacc

    f32 = mybir.dt.float32
    bf16 = mybir.dt.bfloat16
    AF = mybir.ActivationFunctionType

    nc = bacc.Bacc("TRN2", target_bir_lowering=False, debug=False)

    x_d = nc.dram_tensor("x", [N, N], bf16, kind="ExternalInput")
    # atp cols 0:560 y-interleaved (stage 1), 560:1352 j-chunked 99-col
    # stacks [Ar | gap | Ai] (stage 1b single-chain stationary)
    atp_d = nc.dram_tensor("atp", [128, 1352], bf16, kind="ExternalInput")
    # kri: 99-row stacks (rows 0:35 / 64:99) with 12 pair-blocks of 99 cols;
    # cols 0:1188 multiply M_r (Kr-; Ki-stack), cols 1188:2376 multiply M_i.
    # (NCC_IBIR297: SBUF TensorTensor inputs must share a base partition, so
    # the swapped copy cannot be replaced by cross-block operands.)
    kri_d = nc.dram_tensor("kri", [99, 2 * 12 * 99], bf16, kind="ExternalInput")
    # cc = [ctp99 (144) | cc99r (72) | cc99i (72)]  [99, 288]
    cc_d = nc.dram_tensor("cc", [99, 288], bf16, kind="ExternalInput")
    # uc = [uht_h | ut]  [72, 1536]
    uc_d = nc.dram_tensor("uc", [NC, 1536], bf16, kind="ExternalInput")

    aerial_d = nc.dram_tensor("aerial", [512, N], bf16, kind="ExternalOutput")

    with tile.TileContext(nc) as tc:
        with (
            tc.tile_pool(name="const", bufs=1) as cpool,
            tc.tile_pool(name="xin", bufs=8) as xpool,
            tc.tile_pool(name="work", bufs=1) as wpool,
            tc.tile_pool(name="scr", bufs=2) as spool,
            tc.tile_pool(name="sq", bufs=6) as sqpool,
            tc.tile_pool(name="outp", bufs=3) as opool,
        ):
            # ---- input DMAs: x + atp first; kri/cc/uc trail on same rings ----
            # x chunk c holds DRAM rows 8p+2c, 8p+2c+1 on partition p -> the
            # two rows are adjacent in DRAM = 4KB descriptor runs
            x_sb = [xpool.tile([128, N], bf16, tag="x", name=f"x{i}")
                    for i in range(8)]
            xv = x_d.ap().rearrange("(p r) j -> p r j", p=128)
            atp_sb = cpool.tile([128, 1352], bf16)
            kri_sb = cpool.tile([99, 2 * 12 * 99], bf16)
            cc_sb = cpool.tile([99, 288], bf16)
            uc_sb = cpool.tile([NC, 1536], bf16)

            # single ring in strict priority order: one dma_start's
            # descriptors fan out across all 16 DMA engines, so a single ring
            # gives serial arrival (atp first, then x chunk by chunk, consts
            # last) -- pacing stage 1 without consts stealing bandwidth.
            # kri/cc/uc are needed at ~+6/+8/+16us; trailing serially is fine.
            # atp strictly before x: the PE streams atp_sb during stage 1,
            # and concurrent DMA writes into the tile being streamed slow the
            # matmuls ~20% (measured 59ns -> 71ns per 70-col matmul).
            nc.sync.dma_start(atp_sb[:, 0:560], atp_d[:, 0:560])
            for c in range(8):
                nc.sync.dma_start(x_sb[c][:], xv[:, c, :])
            nc.sync.dma_start(atp_sb[:, 560:1352], atp_d[:, 560:1352])
            nc.sync.dma_start(kri_sb[:], kri_d[:, :])
            nc.sync.dma_start(cc_sb[:], cc_d[:, :])
            nc.sync.dma_start(uc_sb[:], uc_d[:, :])

            # early memsets (no input deps; off the critical path)
            mhat99_r = wpool.tile([99, 128], bf16)
            mhat99_i = wpool.tile([99, 128], bf16)
            gt = wpool.tile([99, 12 * 99], bf16)
            w99 = wpool.tile([99, K * NC], bf16)          # [99, 1728]
            nc.vector.memset(mhat99_r[:], 0.0)
            nc.vector.memset(mhat99_i[:], 0.0)
            nc.gpsimd.memset(gt[32:64, :], 0.0)
            nc.gpsimd.memset(w99[32:64, :], 0.0)

            ctp99 = cc_sb[:, 0:144]
            cc99r = cc_sb[:, 144:216]
            cc99i = cc_sb[:, 216:288]
            uht = uc_sb[:, 0:512]
            ut = uc_sb[:, 512:1536]

            # ---- stage 1: P1T[j,u] = sum_y x[y,j] * atp[y,u] ----
            # One gap-free 64-matmul burst (~3.8us) to ramp the PE p-state.
            p1t_sb = wpool.tile([128, 8 * 2 * HK], bf16)      # [128, 560]
            with tc.tile_pool(name="p1ps", bufs=8, space=bass.MemorySpace.PSUM) as p1ps:
                p1t_ps = [p1ps.tile([128, 2 * HK], f32, tag="p1t", name=f"p1t{i}")
                          for i in range(8)]
                for c in range(8):
                    for jc in range(8):
                        nc.tensor.matmul(
                            p1t_ps[jc][:, :],
                            x_sb[c][:, jc * 128:(jc + 1) * 128],
                            atp_sb[:, c * 70:(c + 1) * 70],
                            start=(c == 0), stop=(c == 7),
                        )
                for jc in range(8):
                    if jc % 2 == 0:
                        nc.scalar.copy(p1t_sb[:, jc * 70:(jc + 1) * 70], p1t_ps[jc][:, :])
                    else:
                        nc.vector.tensor_copy(p1t_sb[:, jc * 70:(jc + 1) * 70],
                                              p1t_ps[jc][:, :])

            # NOTE p-state: the PE clock ramps 1.2->2.4GHz after ~3us of
            # continuous busy but throttles back after ~3us hot and does NOT
            # re-ramp even under gap-free load (measured).  Filler matmuls to
            # hold the clock are useless: the Tile scheduler also hoists them.
            if True:
                # ---- stage 1b: MhatT = A @ P1^T (contract over j) ----
                with tc.tile_pool(name="m4ps", bufs=1, space=bass.MemorySpace.PSUM) as m4ps:
                    m4 = m4ps.tile([99, 2 * HK], f32)
                    for jc in range(8):
                        nc.tensor.matmul(m4[:, :],
                                         atp_sb[:, 560 + jc * 99:560 + (jc + 1) * 99],
                                         p1t_sb[:, jc * 70:(jc + 1) * 70],
                                         start=(jc == 0), stop=(jc == 7))
                    m4b_sb = wpool.tile([HK, 2 * HK], f32)
                    nc.scalar.copy(m4b_sb[:], m4[64:99, :])
                    cview = lambda t, pq: t[pq:pq + HK, :].rearrange(
                        "p (c u) -> p c u", c=2)[:, :, 0:HK]   # cols {0:35, 64:99}
                    bcast = lambda ap: ap.unsqueeze(1).broadcast_to([HK, 2, HK])
                    for pq in (0, 64):
                        nc.vector.tensor_sub(cview(mhat99_r, pq),
                                             bcast(m4[0:HK, 0:HK]),
                                             bcast(m4b_sb[:, HK:2 * HK]))
                        nc.vector.tensor_add(cview(mhat99_i, pq),
                                             bcast(m4[0:HK, HK:2 * HK]),
                                             bcast(m4b_sb[:, 0:HK]))

                # ---- stages 2a-2d in 2 super-rounds of 6 pairs each ----
                # sr covers pairs 6sr..6sr+5; psum wp [99,1024] holds 6 slots;
                # w99 block 2sr = lo kernels, 2sr+1 = hi kernels (col within
                # block = z*216 + j*72 + m, slot = 3z+j).  2d group g runs
                # right after super-round g//2's copies -> squares pipeline.
                # psum-read copies cost ~0.43us nearly independent of size:
                # few large copies beat many small ones.
                t1 = spool.tile([99, 12 * 99], bf16, tag="t", name="t1")
                t2 = spool.tile([99, 12 * 99], bf16, tag="t", name="t2")
                r3 = lambda ap, k: ap.rearrange("q (k p) -> q k p", k=k)
                sq = [sqpool.tile([72, 864], bf16, tag="sq", name=f"sq{g}")
                      for g in range(4)]
                asum = [spool.tile([72, 432], bf16, tag=f"a{g}", name=f"a{g}")
                        for g in range(4)]
                offs6 = (0, 144, 288, 512, 656, 800)
                r2v = lambda ap: ap.rearrange("q (z j m) -> q z j m", z=2, j=3)
                z2 = lambda ap: ap.rearrange("p (z c) -> p z c", z=2)

                def d2_group(g, asum_eng):
                    # 2d group g + |F|^2 + per-group presum
                    fp = fps.tile([72, 1024], f32, tag="fp", name=f"fp{g}")
                    nc.tensor.matmul(fp[:, 0:432], cc99r,
                                     w99[:, g * 432:(g + 1) * 432],
                                     start=True, stop=True)
                    nc.tensor.matmul(fp[:, 512:944], cc99i,
                                     w99[:, g * 432:(g + 1) * 432],
                                     start=True, stop=True)
                    fpv = z2(fp[:])[:, :, 0:432]
                    nc.scalar.activation(z2(sq[g][:]), fpv, AF.Square)
                    asum_eng.tensor_add(asum[g][:], sq[g][:, 0:432],
                                        sq[g][:, 432:864])

                with (
                    tc.tile_pool(name="wps", bufs=2, space=bass.MemorySpace.PSUM) as wps,
                    tc.tile_pool(name="fps", bufs=2, space=bass.MemorySpace.PSUM) as fps,
                ):
                    for sr in range(2):
                        # 2a half sr: G for pairs 6sr..6sr+5
                        c0, c1 = sr * 594, (sr + 1) * 594
                        mr_b6 = mhat99_r[:, 0:99].unsqueeze(1).broadcast_to(
                            [99, 6, 99])
                        mi_b6 = mhat99_i[:, 0:99].unsqueeze(1).broadcast_to(
                            [99, 6, 99])
                        nc.vector.tensor_mul(r3(t1[:, c0:c1], 6), mr_b6,
                                             r3(kri_sb[:, c0:c1], 6))
                        nc.vector.tensor_mul(r3(t2[:, c0:c1], 6), mi_b6,
                                             r3(kri_sb[:, 1188 + c0:1188 + c1], 6))
                        nc.vector.tensor_sub(gt[0:HK, c0:c1], t1[0:HK, c0:c1],
                                             t2[0:HK, c0:c1])
                        nc.vector.tensor_add(gt[64:99, c0:c1], t1[64:99, c0:c1],
                                             t2[64:99, c0:c1])

                        # 2c super-round: 6 pair-matmuls into one [99,1024]
                        wp = wps.tile([99, 1024], f32)
                        for j in range(6):
                            pr = sr * 6 + j
                            nc.tensor.matmul(wp[:, offs6[j]:offs6[j] + 144],
                                             gt[:, pr * 99:(pr + 1) * 99],
                                             ctp99, start=True, stop=True)
                        wpv = wp[:].rearrange("q (z c) -> q z c", z=2)[
                            :, :, 0:432].rearrange("q z (j m) -> q z j m", j=3)
                        cl, ch = 2 * sr * 432, (2 * sr + 1) * 432
                        nc.vector.tensor_copy(r2v(w99[0:HK, cl:cl + 432]),
                                              wpv[0:HK, :, :, 0:72])
                        nc.scalar.copy(r2v(w99[64:99, cl:cl + 432]),
                                       wpv[0:HK, :, :, 72:144])
                        nc.scalar.copy(r2v(w99[0:HK, ch:ch + 432]),
                                       wpv[64:99, :, :, 0:72])
                        nc.vector.tensor_copy(r2v(w99[64:99, ch:ch + 432]),
                                              wpv[64:99, :, :, 72:144])

                        # 2d groups for this super-round; early presums on the
                        # otherwise-idle gpsimd, late ones on vector
                        d2_group(2 * sr, nc.vector)
                        d2_group(2 * sr + 1, nc.vector)

                # ---- intensity sum folds -> single pq [72,72] ----
                # fold() is linear: pre-fold S = fold(a0+a1+a2) early, and
                # after the last square only fold(a3) + one add remain.
                fa0 = wpool.tile([72, 432], f32, tag="fa0", name="fa0")
                s3 = wpool.tile([72, 432], f32, tag="s3", name="s3")
                sb = wpool.tile([72, 216], f32, tag="sb", name="sb")
                sc = wpool.tile([72, 72], f32, tag="sc", name="sc")
                sd = wpool.tile([72, 72], bf16, tag="sd", name="sd")
                b3 = wpool.tile([72, 216], f32, tag="b3", name="b3")
                c3 = wpool.tile([72, 72], f32, tag="c3", name="c3")
                d3 = wpool.tile([72, 72], bf16, tag="d3", name="d3")
                nc.vector.tensor_add(fa0[:], asum[0][:], asum[1][:])
                nc.vector.tensor_add(s3[:], fa0[:], asum[2][:])
                nc.vector.tensor_add(sb[:], s3[:, 0:216], s3[:, 216:432])
                nc.vector.tensor_add(sc[:], sb[:, 0:72], sb[:, 72:144])
                nc.vector.tensor_add(sd[:], sc[:], sb[:, 144:216])
                nc.vector.tensor_add(b3[:], asum[3][:, 0:216], asum[3][:, 216:432])
                nc.vector.tensor_add(c3[:], b3[:, 0:72], b3[:, 72:144])
                nc.vector.tensor_add(d3[:], c3[:], b3[:, 144:216])

                # ---- stage 5: aerial_half = U_h @ aer_c @ U^T ----
                z_sb = wpool.tile([72, 512], bf16)
                with tc.tile_pool(name="zps", bufs=1, space=bass.MemorySpace.PSUM) as zps:
                    zp = zps.tile([72, 512], f32)
                    # sd (groups 0-2, ready before the last square) streams
                    # early; d3 accumulates on top -> only one uht stream on
                    # the post-sq3 critical path... but psum accumulation
                    # needs both matmuls in one chain.
                    nc.tensor.matmul(zp[:], sd[:], uht, start=True, stop=False)
                    nc.tensor.matmul(zp[:], d3[:], uht, start=False, stop=True)
                    nc.scalar.copy(z_sb[:, 0:256], zp[:, 0:256])
                    nc.vector.tensor_copy(z_sb[:, 256:512], zp[:, 256:512])

                # uht cols are host-permuted: z col 128*(2*tau+s)+p holds output
                # row 256*tau + 2p + s -> partition p carries 2 adjacent DRAM
                # rows per 256-row tile = 4KB output descriptor runs
                with tc.tile_pool(name="aps", bufs=3, space=bass.MemorySpace.PSUM) as aps:
                    for tau in range(2):
                        aer_sb = opool.tile([128, 2 * N], bf16, tag="out", name="aer_sb")
                        dv = aerial_d[256 * tau:256 * (tau + 1), :].rearrange(
                            "(p s) y -> p s y", s=2)
                        for s in range(2):
                            ap_t = aps.tile([128, N], f32)
                            zc = 256 * tau + 128 * s
                            # half-copies overlap the second matmul; each
                            # half ships as soon as its copy lands
                            nc.tensor.matmul(ap_t[:, 0:512],
                                             z_sb[:, zc:zc + 128],
                                             ut[:, 0:512], start=True, stop=True)
                            nc.scalar.copy(aer_sb[:, s * N:s * N + 512],
                                           ap_t[:, 0:512])
                            nc.tensor.matmul(ap_t[:, 512:1024],
                                             z_sb[:, zc:zc + 128],
                                             ut[:, 512:1024], start=True, stop=True)
                            nc.vector.tensor_copy(
                                aer_sb[:, s * N + 512:(s + 1) * N],
                                ap_t[:, 512:1024])
                            dq = nc.sync if s == 0 else nc.scalar
                            dq.dma_start(dv[:, s, 0:512],
                                         aer_sb[:, s * N:s * N + 512])
                            dq.dma_start(dv[:, s, 512:1024],
                                         aer_sb[:, s * N + 512:(s + 1) * N])

    nc.compile()
    return nc


_CACHE = {}


def _get_program():
    if "nc" not in _CACHE:
        _CACHE["nc"] = _build_program()
    return _CACHE["nc"]


def _prep_inputs(mask, kernels, scales):
    import ml_dtypes
    bf = ml_dtypes.bfloat16

    atp, ctp99, cc99, ut, U = _host_matrices()

    kers = kernels.astype(np.complex128) * np.sqrt(scales.astype(np.float64))[:, None, None]
    ktR = np.ascontiguousarray(
        kers.real.astype(np.float32).transpose(2, 0, 1).reshape(HK, K * HK))
    ktI = np.ascontiguousarray(
        kers.imag.astype(np.float32).transpose(2, 0, 1).reshape(HK, K * HK))
    # 99-row / 99-col pair-block layout: block p holds kernels (2p, 2p+1) at
    # cols 0:35 / 64:99; rows 0:35 multiply M (kA top), rows 64:99 the swap.
    kri = np.zeros((99, 2 * 12 * 99), np.float32)
    for p in range(12):
        for side, k in ((0, 2 * p), (64, 2 * p + 1)):
            c = p * 99 + side
            kri[0:HK, c:c + HK] = ktR[:, k * HK:(k + 1) * HK]        # t1 top: Kr
            kri[64:99, c:c + HK] = ktI[:, k * HK:(k + 1) * HK]       # t1 bot: Ki
            kri[0:HK, 1188 + c:1188 + c + HK] = ktI[:, k * HK:(k + 1) * HK]
            kri[64:99, 1188 + c:1188 + c + HK] = ktR[:, k * HK:(k + 1) * HK]
    kri = kri.astype(bf)
    # atp: cols 0:560 y-interleaved (atp[8p+r, u], stage 1); cols 560:1352
    # j-chunked 99-col stacks [Ar(35) | zeros(29) | Ai(35)] (stage 1b)
    atp_y = atp.reshape(128, 8 * 2 * HK)                         # [128, 560]
    atp_j = atp.reshape(8, 128, 2 * HK).transpose(1, 0, 2)       # [128, 8, 70]
    atp99 = np.zeros((128, 8, 99), np.float32)
    atp99[:, :, 0:HK] = atp_j[:, :, 0:HK]
    atp99[:, :, 64:99] = atp_j[:, :, HK:2 * HK]
    atp = np.ascontiguousarray(
        np.concatenate([atp_y, atp99.reshape(128, 792)], axis=1))
    cc = np.concatenate([ctp99, cc99], axis=1).astype(bf)      # [99, 288]
    # uht cols permuted so stage-5b's z col 128*(2*tau+s)+p holds output row
    # 256*tau + 2p + s (2 adjacent DRAM rows per partition in the output DMA)
    cidx = np.arange(512)
    rperm = 256 * (cidx // 256) + 2 * (cidx % 128) + ((cidx % 256) // 128)
    uh = [np.ascontiguousarray(U[h * 512:(h + 1) * 512, :].T[:, rperm])
          for h in range(2)]
    uc = [np.concatenate([uh[h], ut], axis=1).astype(bf) for h in range(2)]
    atp_bf = atp.astype(bf)
    mask_bf = np.asarray(mask, np.float32).astype(bf)
    return mask_bf, atp_bf, kri, cc, uc


# ---------------------------------------------------------------- entry point
def kernel(mask, kernels, kernels_ct, scales):
    """Full inputs in, full outputs out.  Shards over 8 NeuronCores internally."""
    from concourse.bass_utils import run_bass_kernel_spmd

    kernels = np.asarray(kernels, np.complex64)
    scales = np.asarray(scales, np.float32)
    mask_bf, atp_bf, kri, cc, uc = _prep_inputs(mask, kernels, scales)

    nc = _get_program()
    in_maps = []
    for c in range(8):
        b, h = c // 2, c % 2
        in_maps.append({
            "x": mask_bf[b],
            "atp": atp_bf,
            "kri": kri,
            "cc": cc,
            "uc": uc[h],
        })

    trace = bool(int(os.environ.get("BASS_KERNEL_TRACE", "0")))
    res = run_bass_kernel_spmd(nc, in_maps, core_ids=list(range(8)), trace=trace)
    _CACHE["last_results"] = res

    aerial = np.empty((B, N, N), np.float32)
    for c in range(8):
        b, h = c // 2, c % 2
        aerial[b, h * 512:(h + 1) * 512, :] = \
            np.asarray(res.results[c]["aerial"]).astype(np.float32)
    resist = (1.0 / (1.0 + np.exp(
        -RESIST_STEEPNESS * (aerial.astype(np.float64) - RESIST_THRESHOLD)
    ))).astype(np.float32)
    printed = (aerial > RESIST_THRESHOLD).astype(np.float32)
    return aerial, resist, printed


# revision 35
# speedup vs baseline: 1.0362x; 1.0362x over previous
"""Trainium2 Bass kernel for the SOCS lithography simulator.

Reference math (per batch b):
    aerial = sum_k s_k * | cIFFT2( cFFT2(mask_b) * pad_center(kernels[k]) ) |^2
    resist = sigmoid(50*(aerial - 0.225));  printed = (aerial > 0.225)

Band-limited formulation (see git history of this file for derivation):
    Mhat  = A @ x @ A.T          A = rows 494:529 of the centered DFT matrix
    G_k   = Mhat * (sqrt(s_k) * kernels[k])                 [35,35] cplx
    F_k   = C @ G_k @ C.T        C = inverse-DFT samples at 72 stride-14 pts
    aer_c = sum_k |F_k|^2        exact coarse samples of aerial
    aerial = U @ aer_c @ U.T     U = Re(E pinv(V)) [1024,72]

Optimizations beyond the 50.4us baseline (final: ~47.3us):
  * input DMA on ONE ring in strict priority order: atp_y (stage-1 cols),
    x in 8 chunks of [128,1024] (1 DRAM row per partition, 2KB runs),
    then atp99 / kri / cc / uc.  One dma_start's descriptors fan out over
    all 16 DMA engines, so a single ring = serial arrival: stage 1 starts
    at ~11us and pipelines chunk-by-chunk under the x DMA.
  * atp strictly before x: the PE streams atp_sb during stage 1, and
    concurrent DMA writes into the tile being streamed slow every matmul
    ~20% (59ns -> 71ns per 70-col matmul, measured).
  * stages 2a-2d run in 2 super-rounds of 6 kernel pairs: 2a half ->
    6 pair-matmuls into one [99,1024] psum tile -> 4 copies of 432 cols
    (2 scalar + 2 vector; psum-read copies cost ~0.4-0.6us nearly
    independent of size, so few big copies beat many small ones) ->
    2 x (2d group matmuls + scalar SQUARE + vector presum a[g]).
  * intensity folds exploit linearity: S = fold6(a0+a1+a2) pre-folds
    during the last square; after sq3 only fold6(a3) (3 ops) remains.
    Stage 5a accumulates zp = S@uht (early, hidden) + d3@uht (late).
  * stage-5b psum tiles bufs=3 so block k+2's matmuls don't wait on
    block k's copies; output copies/DMA split in 512-col halves.
Measured and rejected:
  * PE p-state: clock ramps 1.2->2.4GHz after ~3us continuous tensor busy,
    runs hot ~3us, then throttles back and does NOT re-ramp even under
    gap-free load.  Filler matmuls are useless (and the Tile scheduler
    hoists them to the front of the queue anyway).
  * gpsimd tensor_add: ~1us per 432 cols (eff 0.42) vs vector 380ns; only
    memsets and DMA issue belong there.
  * NCC_IBIR297: SBUF TensorTensor inputs must share a base partition -> the
    kri swapped copy cannot be replaced by cross-block operands.
  * NCC_IBVF027: an instruction may read only ONE input from PSUM -- even
    the same tile twice (no DVE square-from-psum; no psum+psum adds).
  * dma_start cannot touch PSUM (SBUF/DRAM only): psum evacuation is a
    fixed DVE/ACT tax (~12k cols/core), the mid-section bottleneck.
  * cross-engine semaphore latency ~0.7us/hop: the serial ladder
    (copies->matmul->square->fold->matmul->copy->DMA) pays it ~12 times.
  * 16 small w99 copies (216 cols) cost 6.9us vs 8 big ones (432) 3.4us.

Hardware rules learned (cost a debug cycle each, do not regress):
  * a start=True matmul clears has_written bits for its whole PSUM bank ->
    concurrent accumulation chains need one bank each; single-shot
    (start+stop) matmuls may share a bank.
  * matmul PSUM output regions must not cross a 2KB bank boundary.
  * engine AP partition offsets must be multiples of 32 (hence the
    0:35 / 64:99 "99-row stack" layout used everywhere).
  * GPSIMD cannot read PSUM; DVE/ACT can read at most one PSUM operand.
  * collective_compute has a ~10us floor per op on this fabric (first one
    ~40us) - pair-wise k/y-splits via collectives do not pay off here.

Sharding: 8 cores; core c handles batch c//2 and output row-half c%2.
Each core runs stages 1-4 for its batch and half of stage 5. No collectives.

Self-contained: shapes/constants hardcoded, no sibling imports.
"""

import os

import numpy as np

N = 1024
B, K, HK = 4, 24, 35
PT = (N - HK) // 2          # 494
NC = 72                     # coarse grid samples (stride 14; >= 69 needed)
NF = 2 * HK - 1             # 69 product frequencies
RESIST_THRESHOLD = 0.225
RESIST_STEEPNESS = 50.0


# ---------------------------------------------------------------- host matrices
def _host_matrices():
    u = np.arange(HK)[:, None]          # 0..34  (centered freq u-18)
    y = np.arange(N)[None, :]
    A = np.exp(-2j * np.pi * ((u + PT - N // 2) * (y - N // 2)) / N)  # [35,1024]
    ym = 14 * np.arange(NC)
    Cs = np.exp(2j * np.pi * ((np.arange(HK)[None, :] - 18)
                              * (ym[:, None] - 512)) / N) / N         # [72,35]
    f = np.arange(-(NF // 2), NF // 2 + 1)
    V = np.exp(2j * np.pi * (f[None, :] * (ym[:, None] - 512)) / N)   # [72,69]
    E = np.exp(2j * np.pi * (f[None, :]
                             * (np.arange(N)[:, None] - 512)) / N)    # [1024,69]
    U = np.ascontiguousarray((E @ np.linalg.pinv(V)).real)            # [1024,72]

    atp = np.empty((N, 2 * HK), np.float32)          # [1024, 70]  A^T packed
    atp[:, :HK] = A.real.T
    atp[:, HK:] = A.imag.T
    ctr = np.ascontiguousarray(Cs.real.T, np.float32)   # [35,72] Ctr[q,m]=ReC[m,q]
    cti = np.ascontiguousarray(Cs.imag.T, np.float32)
    # ctp99: stacked rhs for stage 2c (contract Re/Im of G in one matmul)
    ctp99 = np.zeros((99, 2 * NC), np.float32)
    ctp99[0:35] = np.concatenate([ctr, cti], axis=1)        # top: [ctr | cti]
    ctp99[64:99] = np.concatenate([-cti, ctr], axis=1)      # bot: [-cti | ctr]
    # cc99: stacked stationary for stage 2d. col block 0: Re out, 1: Im out
    cc99 = np.zeros((99, 2 * NC), np.float32)
    cc99[0:35, 0:NC] = ctr
    cc99[64:99, 0:NC] = -cti
    cc99[0:35, NC:2 * NC] = cti
    cc99[64:99, NC:2 * NC] = ctr
    ut = np.ascontiguousarray(U.T, np.float32)          # [72,1024]
    return atp, ctp99, cc99, ut, U.astype(np.float32)


# ---------------------------------------------------------------- bass program
def _build_program():
    import concourse.bass as bass
    import concourse.mybir as mybir
    import concourse.tile as tile
    from concourse import bacc

    f32 = mybir.dt.float32
    bf16 = mybir.dt.bfloat16
    AF = mybir.ActivationFunctionType

    nc = bacc.Bacc("TRN2", target_bir_lowering=False, debug=False)

    x_d = nc.dram_tensor("x", [N, N], bf16, kind="ExternalInput")
    # atp cols 0:560 y-interleaved (stage 1), 560:1352 j-chunked 99-col
    # stacks [Ar | gap | Ai] (stage 1b single-chain stationary)
    atp_d = nc.dram_tensor("atp", [128, 1352], bf16, kind="ExternalInput")
    # kri: 99-row stacks (rows 0:35 / 64:99) with 12 pair-blocks of 99 cols;
    # cols 0:1188 multiply M_r (Kr-; Ki-stack), cols 1188:2376 multiply M_i.
    # (NCC_IBIR297: SBUF TensorTensor inputs must share a base partition, so
    # the swapped copy cannot be replaced by cross-block operands.)
    kri_d = nc.dram_tensor("kri", [99, 2 * 12 * 99], bf16, kind="ExternalInput")
    # cc = [ctp99 (144) | cc99r (72) | cc99i (72)]  [99, 288]
    cc_d = nc.dram_tensor("cc", [99, 288], bf16, kind="ExternalInput")
    # uc = [uht_h | ut]  [72, 1536]
    uc_d = nc.dram_tensor("uc", [NC, 1536], bf16, kind="ExternalInput")

    aerial_d = nc.dram_tensor("aerial", [512, N], bf16, kind="ExternalOutput")

    with tile.TileContext(nc) as tc:
        with (
            tc.tile_pool(name="const", bufs=1) as cpool,
            tc.tile_pool(name="xin", bufs=8) as xpool,
            tc.tile_pool(name="work", bufs=1) as wpool,
            tc.tile_pool(name="scr", bufs=2) as spool,
            tc.tile_pool(name="sq", bufs=6) as sqpool,
            tc.tile_pool(name="outp", bufs=3) as opool,
        ):
            # ---- input DMAs: x + atp first; kri/cc/uc trail on same rings ----
            # x chunk c holds DRAM rows 8p+2c, 8p+2c+1 on partition p -> the
            # two rows are adjacent in DRAM = 4KB descriptor runs
            x_sb = [xpool.tile([128, N], bf16, tag="x", name=f"x{i}")
                    for i in range(8)]
            xv = x_d.ap().rearrange("(p r) j -> p r j", p=128)
            atp_sb = cpool.tile([128, 1352], bf16)
            kri_sb = cpool.tile([99, 2 * 12 * 99], bf16)
            cc_sb = cpool.tile([99, 288], bf16)
            uc_sb = cpool.tile([NC, 1536], bf16)

            # single ring in strict priority order: one dma_start's
            # descriptors fan out across all 16 DMA engines, so a single ring
            # gives serial arrival (atp first, then x chunk by chunk, consts
            # last) -- pacing stage 1 without consts stealing bandwidth.
            # kri/cc/uc are needed at ~+6/+8/+16us; trailing serially is fine.
            # atp strictly before x: the PE streams atp_sb during stage 1,
            # and concurrent DMA writes into the tile being streamed slow the
            # matmuls ~20% (measured 59ns -> 71ns per 70-col matmul).
            nc.sync.dma_start(atp_sb[:, 0:560], atp_d[:, 0:560])
            for c in range(8):
                nc.sync.dma_start(x_sb[c][:], xv[:, c, :])
            nc.sync.dma_start(atp_sb[:, 560:1352], atp_d[:, 560:1352])
            nc.sync.dma_start(kri_sb[:], kri_d[:, :])
            nc.sync.dma_start(cc_sb[:], cc_d[:, :])
            nc.sync.dma_start(uc_sb[:], uc_d[:, :])

            # early memsets (no input deps; off the critical path)
            mhat99_r = wpool.tile([99, 128], bf16)
            mhat99_i = wpool.tile([99, 128], bf16)
            gt = wpool.tile([99, 12 * 99], bf16)
            w99 = wpool.tile([99, K * NC], bf16)          # [99, 1728]
            nc.vector.memset(mhat99_r[:], 0.0)
            nc.vector.memset(mhat99_i[:], 0.0)
            nc.gpsimd.memset(gt[32:64, :], 0.0)
            nc.gpsimd.memset(w99[32:64, :], 0.0)

            ctp99 = cc_sb[:, 0:144]
            cc99r = cc_sb[:, 144:216]
            cc99i = cc_sb[:, 216:288]
            uht = uc_sb[:, 0:512]
            ut = uc_sb[:, 512:1536]

            # ---- stage 1: P1T[j,u] = sum_y x[y,j] * atp[y,u] ----
            # One gap-free 64-matmul burst (~3.8us) to ramp the PE p-state.
            p1t_sb = wpool.tile([128, 8 * 2 * HK], bf16)      # [128, 560]
            with tc.tile_pool(name="p1ps", bufs=8, space=bass.MemorySpace.PSUM) as p1ps:
                p1t_ps = [p1ps.tile([128, 2 * HK], f32, tag="p1t", name=f"p1t{i}")
                          for i in range(8)]
                for c in range(8):
                    for jc in range(8):
                        nc.tensor.matmul(
                            p1t_ps[jc][:, :],
                            x_sb[c][:, jc * 128:(jc + 1) * 128],
                            atp_sb[:, c * 70:(c + 1) * 70],
                            start=(c == 0), stop=(c == 7),
                        )
                for jc in range(8):
                    if jc % 2 == 0:
                        nc.scalar.copy(p1t_sb[:, jc * 70:(jc + 1) * 70], p1t_ps[jc][:, :])
                    else:
                        nc.vector.tensor_copy(p1t_sb[:, jc * 70:(jc + 1) * 70],
                                              p1t_ps[jc][:, :])

            # NOTE p-state: the PE clock ramps 1.2->2.4GHz after ~3us of
            # continuous busy but throttles back after ~3us hot and does NOT
            # re-ramp even under gap-free load (measured).  Filler matmuls to
            # hold the clock are useless: the Tile scheduler also hoists them.
            if True:
                # ---- stage 1b: MhatT = A @ P1^T (contract over j) ----
                with tc.tile_pool(name="m4ps", bufs=1, space=bass.MemorySpace.PSUM) as m4ps:
                    m4 = m4ps.tile([99, 2 * HK], f32)
                    for jc in range(8):
                        nc.tensor.matmul(m4[:, :],
                                         atp_sb[:, 560 + jc * 99:560 + (jc + 1) * 99],
                                         p1t_sb[:, jc * 70:(jc + 1) * 70],
                                         start=(jc == 0), stop=(jc == 7))
                    m4b_sb = wpool.tile([HK, 2 * HK], f32)
                    nc.scalar.copy(m4b_sb[:], m4[64:99, :])
                    cview = lambda t, pq: t[pq:pq + HK, :].rearrange(
                        "p (c u) -> p c u", c=2)[:, :, 0:HK]   # cols {0:35, 64:99}
                    bcast = lambda ap: ap.unsqueeze(1).broadcast_to([HK, 2, HK])
                    for pq in (0, 64):
                        nc.vector.tensor_sub(cview(mhat99_r, pq),
                                             bcast(m4[0:HK, 0:HK]),
                                             bcast(m4b_sb[:, HK:2 * HK]))
                        nc.vector.tensor_add(cview(mhat99_i, pq),
                                             bcast(m4[0:HK, HK:2 * HK]),
                                             bcast(m4b_sb[:, 0:HK]))

                # ---- stages 2a-2d in 2 super-rounds of 6 pairs each ----
                # sr covers pairs 6sr..6sr+5; psum wp [99,1024] holds 6 slots;
                # w99 block 2sr = lo kernels, 2sr+1 = hi kernels (col within
                # block = z*216 + j*72 + m, slot = 3z+j).  2d group g runs
                # right after super-round g//2's copies -> squares pipeline.
                # psum-read copies cost ~0.43us nearly independent of size:
                # few large copies beat many small ones.
                t1 = spool.tile([99, 12 * 99], bf16, tag="t", name="t1")
                t2 = spool.tile([99, 12 * 99], bf16, tag="t", name="t2")
                r3 = lambda ap, k: ap.rearrange("q (k p) -> q k p", k=k)
                sq = [sqpool.tile([72, 864], bf16, tag="sq", name=f"sq{g}")
                      for g in range(4)]
                asum = [spool.tile([72, 432], bf16, tag=f"a{g}", name=f"a{g}")
                        for g in range(4)]
                offs6 = (0, 144, 288, 512, 656, 800)
                r2v = lambda ap: ap.rearrange("q (z j m) -> q z j m", z=2, j=3)
                z2 = lambda ap: ap.rearrange("p (z c) -> p z c", z=2)

                def d2_group(g, asum_eng):
                    # 2d group g + |F|^2 + per-group presum
                    fp = fps.tile([72, 1024], f32, tag="fp", name=f"fp{g}")
                    nc.tensor.matmul(fp[:, 0:432], cc99r,
                                     w99[:, g * 432:(g + 1) * 432],
                                     start=True, stop=True)
                    nc.tensor.matmul(fp[:, 512:944], cc99i,
                                     w99[:, g * 432:(g + 1) * 432],
                                     start=True, stop=True)
                    fpv = z2(fp[:])[:, :, 0:432]
                    nc.scalar.activation(z2(sq[g][:]), fpv, AF.Square)
                    asum_eng.tensor_add(asum[g][:], sq[g][:, 0:432],
                                        sq[g][:, 432:864])

                with (
                    tc.tile_pool(name="wps", bufs=2, space=bass.MemorySpace.PSUM) as wps,
                    tc.tile_pool(name="fps", bufs=2, space=bass.MemorySpace.PSUM) as fps,
                ):
                    for sr in range(2):
                        # 2a half sr: G for pairs 6sr..6sr+5
                        c0, c1 = sr * 594, (sr + 1) * 594
                        mr_b6 = mhat99_r[:, 0:99].unsqueeze(1).broadcast_to(
                            [99, 6, 99])
                        mi_b6 = mhat99_i[:, 0:99].unsqueeze(1).broadcast_to(
                            [99, 6, 99])
                        nc.vector.tensor_mul(r3(t1[:, c0:c1], 6), mr_b6,
                                             r3(kri_sb[:, c0:c1], 6))
                        nc.vector.tensor_mul(r3(t2[:, c0:c1], 6), mi_b6,
                                             r3(kri_sb[:, 1188 + c0:1188 + c1], 6))
                        nc.vector.tensor_sub(gt[0:HK, c0:c1], t1[0:HK, c0:c1],
                                             t2[0:HK, c0:c1])
                        nc.vector.tensor_add(gt[64:99, c0:c1], t1[64:99, c0:c1],
                                             t2[64:99, c0:c1])

                        # 2c super-round: 6 pair-matmuls into one [99,1024]
                        wp = wps.tile([99, 1024], f32)
                        for j in range(6):
                            pr = sr * 6 + j
                            nc.tensor.matmul(wp[:, offs6[j]:offs6[j] + 144],
                                             gt[:, pr * 99:(pr + 1) * 99],
                                             ctp99, start=True, stop=True)
                        wpv = wp[:].rearrange("q (z c) -> q z c", z=2)[
                            :, :, 0:432].rearrange("q z (j m) -> q z j m", j=3)
                        cl, ch = 2 * sr * 432, (2 * sr + 1) * 432
                        # vector owns BOTH lo copies, scalar BOTH hi:
                        # each 2d group then waits on a single in-order
                        # engine instead of a cross-engine pair.
                        nc.vector.tensor_copy(r2v(w99[0:HK, cl:cl + 432]),
                                              wpv[0:HK, :, :, 0:72])
                        nc.vector.tensor_copy(r2v(w99[64:99, cl:cl + 432]),
                                              wpv[0:HK, :, :, 72:144])
                        nc.scalar.copy(r2v(w99[0:HK, ch:ch + 432]),
                                       wpv[64:99, :, :, 0:72])
                        nc.scalar.copy(r2v(w99[64:99, ch:ch + 432]),
                                       wpv[64:99, :, :, 72:144])

                        # 2d groups for this super-round; early presums on the
                        # otherwise-idle gpsimd, late ones on vector
                        d2_group(2 * sr, nc.vector)
                        d2_group(2 * sr + 1, nc.vector)

                # ---- intensity sum folds -> single pq [72,72] ----
                # fold() is linear: pre-fold S = fold(a0+a1+a2) early, and
                # after the last square only fold(a3) + one add remain.
                fa0 = wpool.tile([72, 432], f32, tag="fa0", name="fa0")
                s3 = wpool.tile([72, 432], f32, tag="s3", name="s3")
                sb = wpool.tile([72, 216], f32, tag="sb", name="sb")
                sc = wpool.tile([72, 72], f32, tag="sc", name="sc")
                sd = wpool.tile([72, 72], bf16, tag="sd", name="sd")
                b3 = wpool.tile([72, 216], f32, tag="b3", name="b3")
                c3 = wpool.tile([72, 72], f32, tag="c3", name="c3")
                d3 = wpool.tile([72, 72], bf16, tag="d3", name="d3")
                nc.vector.tensor_add(fa0[:], asum[0][:], asum[1][:])
                nc.vector.tensor_add(s3[:], fa0[:], asum[2][:])
                nc.vector.tensor_add(sb[:], s3[:, 0:216], s3[:, 216:432])
                nc.vector.tensor_add(sc[:], sb[:, 0:72], sb[:, 72:144])
                nc.vector.tensor_add(sd[:], sc[:], sb[:, 144:216])
                nc.vector.tensor_add(b3[:], asum[3][:, 0:216], asum[3][:, 216:432])
                nc.vector.tensor_add(c3[:], b3[:, 0:72], b3[:, 72:144])
                nc.vector.tensor_add(d3[:], c3[:], b3[:, 144:216])

                # ---- stage 5: aerial_half = U_h @ aer_c @ U^T ----
                z_sb = wpool.tile([72, 512], bf16)
                with tc.tile_pool(name="zps", bufs=1, space=bass.MemorySpace.PSUM) as zps:
                    zp = zps.tile([72, 512], f32)
                    # sd (groups 0-2, ready before the last square) streams
                    # early; d3 accumulates on top -> only one uht stream on
                    # the post-sq3 critical path... but psum accumulation
                    # needs both matmuls in one chain.
                    nc.tensor.matmul(zp[:], sd[:], uht, start=True, stop=False)
                    nc.tensor.matmul(zp[:], d3[:], uht, start=False, stop=True)
                    nc.scalar.copy(z_sb[:, 0:256], zp[:, 0:256])
                    nc.vector.tensor_copy(z_sb[:, 256:512], zp[:, 256:512])

                # uht cols are host-permuted: z col 128*(2*tau+s)+p holds output
                # row 256*tau + 2p + s -> partition p carries 2 adjacent DRAM
                # rows per 256-row tile = 4KB output descriptor runs
                with tc.tile_pool(name="aps", bufs=3, space=bass.MemorySpace.PSUM) as aps:
                    for tau in range(2):
                        aer_sb = opool.tile([128, 2 * N], bf16, tag="out", name="aer_sb")
                        dv = aerial_d[256 * tau:256 * (tau + 1), :].rearrange(
                            "(p s) y -> p s y", s=2)
                        for s in range(2):
                            ap_t = aps.tile([128, N], f32)
                            zc = 256 * tau + 128 * s
                            # half-copies overlap the second matmul; each
                            # half ships as soon as its copy lands
                            nc.tensor.matmul(ap_t[:, 0:512],
                                             z_sb[:, zc:zc + 128],
                                             ut[:, 0:512], start=True, stop=True)
                            nc.scalar.copy(aer_sb[:, s * N:s * N + 512],
                                           ap_t[:, 0:512])
                            nc.tensor.matmul(ap_t[:, 512:1024],
                                             z_sb[:, zc:zc + 128],
                                             ut[:, 512:1024], start=True, stop=True)
                            nc.vector.tensor_copy(
                                aer_sb[:, s * N + 512:(s + 1) * N],
                                ap_t[:, 512:1024])
                            dq = nc.sync if s == 0 else nc.scalar
                            dq.dma_start(dv[:, s, 0:512],
                                         aer_sb[:, s * N:s * N + 512])
                            dq.dma_start(dv[:, s, 512:1024],
                                         aer_sb[:, s * N + 512:(s + 1) * N])

    nc.compile()
    return nc


_CACHE = {}


def _get_program():
    if "nc" not in _CACHE:
        _CACHE["nc"] = _build_program()
    return _CACHE["nc"]


def _prep_inputs(mask, kernels, scales):
    import ml_dtypes
    bf = ml_dtypes.bfloat16

    atp, ctp99, cc99, ut, U = _host_matrices()

    kers = kernels.astype(np.complex128) * np.sqrt(scales.astype(np.float64))[:, None, None]
    ktR = np.ascontiguousarray(
        kers.real.astype(np.float32).transpose(2, 0, 1).reshape(HK, K * HK))
    ktI = np.ascontiguousarray(
        kers.imag.astype(np.float32).transpose(2, 0, 1).reshape(HK, K * HK))
    # 99-row / 99-col pair-block layout: block p holds kernels (2p, 2p+1) at
    # cols 0:35 / 64:99; rows 0:35 multiply M (kA top), rows 64:99 the swap.
    kri = np.zeros((99, 2 * 12 * 99), np.float32)
    for p in range(12):
        for side, k in ((0, 2 * p), (64, 2 * p + 1)):
            c = p * 99 + side
            kri[0:HK, c:c + HK] = ktR[:, k * HK:(k + 1) * HK]        # t1 top: Kr
            kri[64:99, c:c + HK] = ktI[:, k * HK:(k + 1) * HK]       # t1 bot: Ki
            kri[0:HK, 1188 + c:1188 + c + HK] = ktI[:, k * HK:(k + 1) * HK]
            kri[64:99, 1188 + c:1188 + c + HK] = ktR[:, k * HK:(k + 1) * HK]
    kri = kri.astype(bf)
    # atp: cols 0:560 y-interleaved (atp[8p+r, u], stage 1); cols 560:1352
    # j-chunked 99-col stacks [Ar(35) | zeros(29) | Ai(35)] (stage 1b)
    atp_y = atp.reshape(128, 8 * 2 * HK)                         # [128, 560]
    atp_j = atp.reshape(8, 128, 2 * HK).transpose(1, 0, 2)       # [128, 8, 70]
    atp99 = np.zeros((128, 8, 99), np.float32)
    atp99[:, :, 0:HK] = atp_j[:, :, 0:HK]
    atp99[:, :, 64:99] = atp_j[:, :, HK:2 * HK]
    atp = np.ascontiguousarray(
        np.concatenate([atp_y, atp99.reshape(128, 792)], axis=1))
    cc = np.concatenate([ctp99, cc99], axis=1).astype(bf)      # [99, 288]
    # uht cols permuted so stage-5b's z col 128*(2*tau+s)+p holds output row
    # 256*tau + 2p + s (2 adjacent DRAM rows per partition in the output DMA)
    cidx = np.arange(512)
    rperm = 256 * (cidx // 256) + 2 * (cidx % 128) + ((cidx % 256) // 128)
    uh = [np.ascontiguousarray(U[h * 512:(h + 1) * 512, :].T[:, rperm])
          for h in range(2)]
    uc = [np.concatenate([uh[h], ut], axis=1).astype(bf) for h in range(2)]
    atp_bf = atp.astype(bf)
    mask_bf = np.asarray(mask, np.float32).astype(bf)
    return mask_bf, atp_bf, kri, cc, uc


# ---------------------------------------------------------------- entry point
def kernel(mask, kernels, kernels_ct, scales):
    """Full inputs in, full outputs out.  Shards over 8 NeuronCores internally."""
    from concourse.bass_utils import run_bass_kernel_spmd

    kernels = np.asarray(kernels, np.complex64)
    scales = np.asarray(scales, np.float32)
    mask_bf, atp_bf, kri, cc, uc = _prep_inputs(mask, kernels, scales)

    nc = _get_program()
    in_maps = []
    for c in range(8):
        b, h = c // 2, c % 2
        in_maps.append({
            "x": mask_bf[b],
            "atp": atp_bf,
            "kri": kri,
            "cc": cc,
            "uc": uc[h],
        })

    trace = bool(int(os.environ.get("BASS_KERNEL_TRACE", "0")))
    res = run_bass_kernel_spmd(nc, in_maps, core_ids=list(range(8)), trace=trace)
    _CACHE["last_results"] = res

    aerial = np.empty((B, N, N), np.float32)
    for c in range(8):
        b, h = c // 2, c % 2
        aerial[b, h * 512:(h + 1) * 512, :] = \
            np.asarray(res.results[c]["aerial"]).astype(np.float32)
    resist = (1.0 / (1.0 + np.exp(
        -RESIST_STEEPNESS * (aerial.astype(np.float64) - RESIST_THRESHOLD)
    ))).astype(np.float32)
    printed = (aerial > RESIST_THRESHOLD).astype(np.float32)
    return aerial, resist, printed


# revision 36
# speedup vs baseline: 1.0593x; 1.0223x over previous
"""Trainium2 Bass kernel for the SOCS lithography simulator.

Reference math (per batch b):
    aerial = sum_k s_k * | cIFFT2( cFFT2(mask_b) * pad_center(kernels[k]) ) |^2
    resist = sigmoid(50*(aerial - 0.225));  printed = (aerial > 0.225)

Band-limited formulation (see git history of this file for derivation):
    Mhat  = A @ x @ A.T          A = rows 494:529 of the centered DFT matrix
    G_k   = Mhat * (sqrt(s_k) * kernels[k])                 [35,35] cplx
    F_k   = C @ G_k @ C.T        C = inverse-DFT samples at 72 stride-14 pts
    aer_c = sum_k |F_k|^2        exact coarse samples of aerial
    aerial = U @ aer_c @ U.T     U = Re(E pinv(V)) [1024,72]

Optimizations beyond the 50.4us baseline (final: ~47.3us):
  * input DMA on ONE ring in strict priority order: atp_y (stage-1 cols),
    x in 8 chunks of [128,1024] (1 DRAM row per partition, 2KB runs),
    then atp99 / kri / cc / uc.  One dma_start's descriptors fan out over
    all 16 DMA engines, so a single ring = serial arrival: stage 1 starts
    at ~11us and pipelines chunk-by-chunk under the x DMA.
  * atp strictly before x: the PE streams atp_sb during stage 1, and
    concurrent DMA writes into the tile being streamed slow every matmul
    ~20% (59ns -> 71ns per 70-col matmul, measured).
  * stages 2a-2d run in 2 super-rounds of 6 kernel pairs: 2a half ->
    6 pair-matmuls into one [99,1024] psum tile -> 4 copies of 432 cols
    (2 scalar + 2 vector; psum-read copies cost ~0.4-0.6us nearly
    independent of size, so few big copies beat many small ones) ->
    2 x (2d group matmuls + scalar SQUARE + vector presum a[g]).
  * intensity folds exploit linearity: S = fold6(a0+a1+a2) pre-folds
    during the last square; after sq3 only fold6(a3) (3 ops) remains.
    Stage 5a accumulates zp = S@uht (early, hidden) + d3@uht (late).
  * stage-5b psum tiles bufs=3 so block k+2's matmuls don't wait on
    block k's copies; output copies/DMA split in 512-col halves.
Measured and rejected:
  * PE p-state: clock ramps 1.2->2.4GHz after ~3us continuous tensor busy,
    runs hot ~3us, then throttles back and does NOT re-ramp even under
    gap-free load.  Filler matmuls are useless (and the Tile scheduler
    hoists them to the front of the queue anyway).
  * gpsimd tensor_add: ~1us per 432 cols (eff 0.42) vs vector 380ns; only
    memsets and DMA issue belong there.
  * NCC_IBIR297: SBUF TensorTensor inputs must share a base partition -> the
    kri swapped copy cannot be replaced by cross-block operands.
  * NCC_IBVF027: an instruction may read only ONE input from PSUM -- even
    the same tile twice (no DVE square-from-psum; no psum+psum adds).
  * dma_start cannot touch PSUM (SBUF/DRAM only): psum evacuation is a
    fixed DVE/ACT tax (~12k cols/core), the mid-section bottleneck.
  * cross-engine semaphore latency ~0.7us/hop: the serial ladder
    (copies->matmul->square->fold->matmul->copy->DMA) pays it ~12 times.
  * 16 small w99 copies (216 cols) cost 6.9us vs 8 big ones (432) 3.4us.

Hardware rules learned (cost a debug cycle each, do not regress):
  * a start=True matmul clears has_written bits for its whole PSUM bank ->
    concurrent accumulation chains need one bank each; single-shot
    (start+stop) matmuls may share a bank.
  * matmul PSUM output regions must not cross a 2KB bank boundary.
  * engine AP partition offsets must be multiples of 32 (hence the
    0:35 / 64:99 "99-row stack" layout used everywhere).
  * GPSIMD cannot read PSUM; DVE/ACT can read at most one PSUM operand.
  * collective_compute has a ~10us floor per op on this fabric (first one
    ~40us) - pair-wise k/y-splits via collectives do not pay off here.

Sharding: 8 cores; core c handles batch c//2 and output row-half c%2.
Each core runs stages 1-4 for its batch and half of stage 5. No collectives.

Self-contained: shapes/constants hardcoded, no sibling imports.
"""

import os

import numpy as np

N = 1024
B, K, HK = 4, 24, 35
PT = (N - HK) // 2          # 494
NC = 72                     # coarse grid samples (stride 14; >= 69 needed)
NF = 2 * HK - 1             # 69 product frequencies
RESIST_THRESHOLD = 0.225
RESIST_STEEPNESS = 50.0


# ---------------------------------------------------------------- host matrices
def _host_matrices():
    u = np.arange(HK)[:, None]          # 0..34  (centered freq u-18)
    y = np.arange(N)[None, :]
    A = np.exp(-2j * np.pi * ((u + PT - N // 2) * (y - N // 2)) / N)  # [35,1024]
    ym = 14 * np.arange(NC)
    Cs = np.exp(2j * np.pi * ((np.arange(HK)[None, :] - 18)
                              * (ym[:, None] - 512)) / N) / N         # [72,35]
    f = np.arange(-(NF // 2), NF // 2 + 1)
    V = np.exp(2j * np.pi * (f[None, :] * (ym[:, None] - 512)) / N)   # [72,69]
    E = np.exp(2j * np.pi * (f[None, :]
                             * (np.arange(N)[:, None] - 512)) / N)    # [1024,69]
    U = np.ascontiguousarray((E @ np.linalg.pinv(V)).real)            # [1024,72]

    atp = np.empty((N, 2 * HK), np.float32)          # [1024, 70]  A^T packed
    atp[:, :HK] = A.real.T
    atp[:, HK:] = A.imag.T
    ctr = np.ascontiguousarray(Cs.real.T, np.float32)   # [35,72] Ctr[q,m]=ReC[m,q]
    cti = np.ascontiguousarray(Cs.imag.T, np.float32)
    # ctp99: stacked rhs for stage 2c (contract Re/Im of G in one matmul)
    ctp99 = np.zeros((99, 2 * NC), np.float32)
    ctp99[0:35] = np.concatenate([ctr, cti], axis=1)        # top: [ctr | cti]
    ctp99[64:99] = np.concatenate([-cti, ctr], axis=1)      # bot: [-cti | ctr]
    # cc99: stacked stationary for stage 2d. col block 0: Re out, 1: Im out
    cc99 = np.zeros((99, 2 * NC), np.float32)
    cc99[0:35, 0:NC] = ctr
    cc99[64:99, 0:NC] = -cti
    cc99[0:35, NC:2 * NC] = cti
    cc99[64:99, NC:2 * NC] = ctr
    ut = np.ascontiguousarray(U.T, np.float32)          # [72,1024]
    return atp, ctp99, cc99, ut, U.astype(np.float32)


# ---------------------------------------------------------------- bass program
def _build_program():
    import concourse.bass as bass
    import concourse.mybir as mybir
    import concourse.tile as tile
    from concourse import bacc

    f32 = mybir.dt.float32
    bf16 = mybir.dt.bfloat16
    AF = mybir.ActivationFunctionType

    nc = bacc.Bacc("TRN2", target_bir_lowering=False, debug=False)

    x_d = nc.dram_tensor("x", [N, N], bf16, kind="ExternalInput")
    # atp cols 0:560 y-interleaved (stage 1), 560:1352 j-chunked 99-col
    # stacks [Ar | gap | Ai] (stage 1b single-chain stationary)
    atp_d = nc.dram_tensor("atp", [128, 1352], bf16, kind="ExternalInput")
    # kri: 99-row stacks (rows 0:35 / 64:99) with 12 pair-blocks of 99 cols;
    # cols 0:1188 multiply M_r (Kr-; Ki-stack), cols 1188:2376 multiply M_i.
    # (NCC_IBIR297: SBUF TensorTensor inputs must share a base partition, so
    # the swapped copy cannot be replaced by cross-block operands.)
    kri_d = nc.dram_tensor("kri", [99, 2 * 12 * 99], bf16, kind="ExternalInput")
    # cc = [ctp99 (144) | cc99r (72) | cc99i (72)]  [99, 288]
    cc_d = nc.dram_tensor("cc", [99, 288], bf16, kind="ExternalInput")
    # uc = [uht_h | ut]  [72, 1536]
    uc_d = nc.dram_tensor("uc", [NC, 1536], bf16, kind="ExternalInput")

    aerial_d = nc.dram_tensor("aerial", [512, N], bf16, kind="ExternalOutput")

    with tile.TileContext(nc) as tc:
        with (
            tc.tile_pool(name="const", bufs=1) as cpool,
            tc.tile_pool(name="xin", bufs=8) as xpool,
            tc.tile_pool(name="work", bufs=1) as wpool,
            tc.tile_pool(name="scr", bufs=2) as spool,
            tc.tile_pool(name="sq", bufs=6) as sqpool,
            tc.tile_pool(name="outp", bufs=3) as opool,
        ):
            # ---- input DMAs: x + atp first; kri/cc/uc trail on same rings ----
            # x chunk c holds DRAM rows 8p+2c, 8p+2c+1 on partition p -> the
            # two rows are adjacent in DRAM = 4KB descriptor runs
            x_sb = [xpool.tile([128, N], bf16, tag="x", name=f"x{i}")
                    for i in range(8)]
            xv = x_d.ap().rearrange("(p r) j -> p r j", p=128)
            atp_sb = cpool.tile([128, 1352], bf16)
            kri_sb = cpool.tile([99, 2 * 12 * 99], bf16)
            cc_sb = cpool.tile([99, 288], bf16)
            uc_sb = cpool.tile([NC, 1536], bf16)

            # single ring in strict priority order: one dma_start's
            # descriptors fan out across all 16 DMA engines, so a single ring
            # gives serial arrival (atp first, then x chunk by chunk, consts
            # last) -- pacing stage 1 without consts stealing bandwidth.
            # kri/cc/uc are needed at ~+6/+8/+16us; trailing serially is fine.
            # atp strictly before x: the PE streams atp_sb during stage 1,
            # and concurrent DMA writes into the tile being streamed slow the
            # matmuls ~20% (measured 59ns -> 71ns per 70-col matmul).
            nc.sync.dma_start(atp_sb[:, 0:560], atp_d[:, 0:560])
            for c in range(8):
                nc.sync.dma_start(x_sb[c][:], xv[:, c, :])
            nc.sync.dma_start(atp_sb[:, 560:1352], atp_d[:, 560:1352])
            nc.sync.dma_start(kri_sb[:], kri_d[:, :])
            nc.sync.dma_start(cc_sb[:], cc_d[:, :])
            nc.sync.dma_start(uc_sb[:], uc_d[:, :])

            # early memsets (no input deps; off the critical path)
            mhat99_r = wpool.tile([99, 128], bf16)
            mhat99_i = wpool.tile([99, 128], bf16)
            gt = wpool.tile([99, 12 * 99], bf16)
            w99 = wpool.tile([99, K * NC], bf16)          # [99, 1728]
            nc.vector.memset(mhat99_r[:], 0.0)
            nc.vector.memset(mhat99_i[:], 0.0)
            nc.gpsimd.memset(gt[32:64, :], 0.0)
            nc.gpsimd.memset(w99[32:64, :], 0.0)

            ctp99 = cc_sb[:, 0:144]
            cc99r = cc_sb[:, 144:216]
            cc99i = cc_sb[:, 216:288]
            uht = uc_sb[:, 0:512]
            ut = uc_sb[:, 512:1536]

            # ---- stage 1: P1T[j,u] = sum_y x[y,j] * atp[y,u] ----
            # One gap-free 64-matmul burst (~3.8us) to ramp the PE p-state.
            p1t_sb = wpool.tile([128, 8 * 2 * HK], bf16)      # [128, 560]
            with tc.tile_pool(name="p1ps", bufs=8, space=bass.MemorySpace.PSUM) as p1ps:
                p1t_ps = [p1ps.tile([128, 2 * HK], f32, tag="p1t", name=f"p1t{i}")
                          for i in range(8)]
                for c in range(8):
                    for jc in range(8):
                        nc.tensor.matmul(
                            p1t_ps[jc][:, :],
                            x_sb[c][:, jc * 128:(jc + 1) * 128],
                            atp_sb[:, c * 70:(c + 1) * 70],
                            start=(c == 0), stop=(c == 7),
                        )
                for jc in range(8):
                    if jc % 2 == 0:
                        nc.scalar.copy(p1t_sb[:, jc * 70:(jc + 1) * 70], p1t_ps[jc][:, :])
                    else:
                        nc.vector.tensor_copy(p1t_sb[:, jc * 70:(jc + 1) * 70],
                                              p1t_ps[jc][:, :])

            # NOTE p-state: the PE clock ramps 1.2->2.4GHz after ~3us of
            # continuous busy but throttles back after ~3us hot and does NOT
            # re-ramp even under gap-free load (measured).  Filler matmuls to
            # hold the clock are useless: the Tile scheduler also hoists them.
            if True:
                # ---- stage 1b: MhatT = A @ P1^T (contract over j) ----
                with tc.tile_pool(name="m4ps", bufs=1, space=bass.MemorySpace.PSUM) as m4ps:
                    m4 = m4ps.tile([99, 2 * HK], f32)
                    for jc in range(8):
                        nc.tensor.matmul(m4[:, :],
                                         atp_sb[:, 560 + jc * 99:560 + (jc + 1) * 99],
                                         p1t_sb[:, jc * 70:(jc + 1) * 70],
                                         start=(jc == 0), stop=(jc == 7))
                    m4b_sb = wpool.tile([HK, 2 * HK], f32)
                    nc.scalar.copy(m4b_sb[:], m4[64:99, :])
                    cview = lambda t, pq: t[pq:pq + HK, :].rearrange(
                        "p (c u) -> p c u", c=2)[:, :, 0:HK]   # cols {0:35, 64:99}
                    bcast = lambda ap: ap.unsqueeze(1).broadcast_to([HK, 2, HK])
                    for pq in (0, 64):
                        nc.vector.tensor_sub(cview(mhat99_r, pq),
                                             bcast(m4[0:HK, 0:HK]),
                                             bcast(m4b_sb[:, HK:2 * HK]))
                        nc.vector.tensor_add(cview(mhat99_i, pq),
                                             bcast(m4[0:HK, HK:2 * HK]),
                                             bcast(m4b_sb[:, 0:HK]))

                # ---- stages 2a-2d in 2 super-rounds of 6 pairs each ----
                # sr covers pairs 6sr..6sr+5; psum wp [99,1024] holds 6 slots;
                # w99 block 2sr = lo kernels, 2sr+1 = hi kernels (col within
                # block = z*216 + j*72 + m, slot = 3z+j).  2d group g runs
                # right after super-round g//2's copies -> squares pipeline.
                # psum-read copies cost ~0.43us nearly independent of size:
                # few large copies beat many small ones.
                t1 = spool.tile([99, 12 * 99], bf16, tag="t", name="t1")
                t2 = spool.tile([99, 12 * 99], bf16, tag="t", name="t2")
                r3 = lambda ap, k: ap.rearrange("q (k p) -> q k p", k=k)
                sq = [sqpool.tile([72, 864], bf16, tag="sq", name=f"sq{g}")
                      for g in range(4)]
                asum = [spool.tile([72, 432], bf16, tag=f"a{g}", name=f"a{g}")
                        for g in range(4)]
                offs6 = (0, 144, 288, 512, 656, 800)
                r2v = lambda ap: ap.rearrange("q (z j m) -> q z j m", z=2, j=3)
                z2 = lambda ap: ap.rearrange("p (z c) -> p z c", z=2)

                def d2_group(g, asum_eng):
                    # 2d group g + |F|^2 + per-group presum
                    fp = fps.tile([72, 1024], f32, tag="fp", name=f"fp{g}")
                    nc.tensor.matmul(fp[:, 0:432], cc99r,
                                     w99[:, g * 432:(g + 1) * 432],
                                     start=True, stop=True)
                    nc.tensor.matmul(fp[:, 512:944], cc99i,
                                     w99[:, g * 432:(g + 1) * 432],
                                     start=True, stop=True)
                    fpv = z2(fp[:])[:, :, 0:432]
                    nc.scalar.activation(z2(sq[g][:]), fpv, AF.Square)
                    asum_eng.tensor_add(asum[g][:], sq[g][:, 0:432],
                                        sq[g][:, 432:864])

                with (
                    tc.tile_pool(name="wps", bufs=2, space=bass.MemorySpace.PSUM) as wps,
                    tc.tile_pool(name="fps", bufs=2, space=bass.MemorySpace.PSUM) as fps,
                ):
                    for sr in range(2):
                        # 2a half sr: G for pairs 6sr..6sr+5
                        c0, c1 = sr * 594, (sr + 1) * 594
                        mr_b6 = mhat99_r[:, 0:99].unsqueeze(1).broadcast_to(
                            [99, 6, 99])
                        mi_b6 = mhat99_i[:, 0:99].unsqueeze(1).broadcast_to(
                            [99, 6, 99])
                        nc.vector.tensor_mul(r3(t1[:, c0:c1], 6), mr_b6,
                                             r3(kri_sb[:, c0:c1], 6))
                        nc.vector.tensor_mul(r3(t2[:, c0:c1], 6), mi_b6,
                                             r3(kri_sb[:, 1188 + c0:1188 + c1], 6))
                        nc.vector.tensor_sub(gt[0:HK, c0:c1], t1[0:HK, c0:c1],
                                             t2[0:HK, c0:c1])
                        nc.vector.tensor_add(gt[64:99, c0:c1], t1[64:99, c0:c1],
                                             t2[64:99, c0:c1])

                        # 2c super-round: 6 pair-matmuls into one [99,1024]
                        wp = wps.tile([99, 1024], f32)
                        for j in range(6):
                            pr = sr * 6 + j
                            nc.tensor.matmul(wp[:, offs6[j]:offs6[j] + 144],
                                             gt[:, pr * 99:(pr + 1) * 99],
                                             ctp99, start=True, stop=True)
                        wpv = wp[:].rearrange("q (z c) -> q z c", z=2)[
                            :, :, 0:432].rearrange("q z (j m) -> q z j m", j=3)
                        cl, ch = 2 * sr * 432, (2 * sr + 1) * 432
                        nc.vector.tensor_copy(r2v(w99[0:HK, cl:cl + 432]),
                                              wpv[0:HK, :, :, 0:72])
                        nc.scalar.copy(r2v(w99[64:99, cl:cl + 432]),
                                       wpv[0:HK, :, :, 72:144])
                        nc.scalar.copy(r2v(w99[0:HK, ch:ch + 432]),
                                       wpv[64:99, :, :, 0:72])
                        nc.vector.tensor_copy(r2v(w99[64:99, ch:ch + 432]),
                                              wpv[64:99, :, :, 72:144])

                        # 2d groups for this super-round; early presums on the
                        # otherwise-idle gpsimd, late ones on vector
                        d2_group(2 * sr, nc.vector)
                        d2_group(2 * sr + 1, nc.vector)

                # ---- intensity sum folds -> single pq [72,72] ----
                # fold() is linear: pre-fold S = fold(a0+a1+a2) early, and
                # after the last square only fold(a3) + one add remain.
                fa0 = wpool.tile([72, 432], f32, tag="fa0", name="fa0")
                s3 = wpool.tile([72, 432], f32, tag="s3", name="s3")
                sb = wpool.tile([72, 216], f32, tag="sb", name="sb")
                sc = wpool.tile([72, 72], f32, tag="sc", name="sc")
                sd = wpool.tile([72, 72], bf16, tag="sd", name="sd")
                b3 = wpool.tile([72, 216], f32, tag="b3", name="b3")
                c3 = wpool.tile([72, 72], f32, tag="c3", name="c3")
                d3 = wpool.tile([72, 72], bf16, tag="d3", name="d3")
                nc.vector.tensor_add(fa0[:], asum[0][:], asum[1][:])
                nc.vector.tensor_add(s3[:], fa0[:], asum[2][:])
                nc.vector.tensor_add(sb[:], s3[:, 0:216], s3[:, 216:432])
                nc.vector.tensor_add(sc[:], sb[:, 0:72], sb[:, 72:144])
                nc.vector.tensor_add(sd[:], sc[:], sb[:, 144:216])
                nc.vector.tensor_add(b3[:], asum[3][:, 0:216], asum[3][:, 216:432])
                nc.vector.tensor_add(c3[:], b3[:, 0:72], b3[:, 72:144])
                nc.vector.tensor_add(d3[:], c3[:], b3[:, 144:216])

                # ---- stage 5: aerial_half = U_h @ aer_c @ U^T ----
                z_sb = wpool.tile([72, 512], bf16)
                with tc.tile_pool(name="zps", bufs=1, space=bass.MemorySpace.PSUM) as zps:
                    zp = zps.tile([72, 512], f32)
                    # sd (groups 0-2, ready before the last square) streams
                    # early; d3 accumulates on top -> only one uht stream on
                    # the post-sq3 critical path... but psum accumulation
                    # needs both matmuls in one chain.
                    nc.tensor.matmul(zp[:], sd[:], uht, start=True, stop=False)
                    nc.tensor.matmul(zp[:], d3[:], uht, start=False, stop=True)
                    nc.scalar.copy(z_sb[:, 0:256], zp[:, 0:256])
                    nc.vector.tensor_copy(z_sb[:, 256:512], zp[:, 256:512])

                # uht cols are host-permuted: z col 128*(2*tau+s)+p holds output
                # row 256*tau + 2p + s -> partition p carries 2 adjacent DRAM
                # rows per 256-row tile = 4KB output descriptor runs
                with tc.tile_pool(name="aps", bufs=3, space=bass.MemorySpace.PSUM) as aps:
                    for tau in range(2):
                        aer_sb = opool.tile([128, 2 * N], bf16, tag="out", name="aer_sb")
                        dv = aerial_d[256 * tau:256 * (tau + 1), :].rearrange(
                            "(p s) y -> p s y", s=2)
                        for s in range(2):
                            ap_t = aps.tile([128, N], f32)
                            zc = 256 * tau + 128 * s
                            # half-copies overlap the second matmul; each
                            # half ships as soon as its copy lands
                            nc.tensor.matmul(ap_t[:, 0:512],
                                             z_sb[:, zc:zc + 128],
                                             ut[:, 0:512], start=True, stop=True)
                            nc.scalar.copy(aer_sb[:, s * N:s * N + 512],
                                           ap_t[:, 0:512])
                            nc.tensor.matmul(ap_t[:, 512:1024],
                                             z_sb[:, zc:zc + 128],
                                             ut[:, 512:1024], start=True, stop=True)
                            nc.vector.tensor_copy(
                                aer_sb[:, s * N + 512:(s + 1) * N],
                                ap_t[:, 512:1024])
                            dq = nc.sync if s == 0 else nc.scalar
                            dq.dma_start(dv[:, s, 0:512],
                                         aer_sb[:, s * N:s * N + 512])
                            dq.dma_start(dv[:, s, 512:1024],
                                         aer_sb[:, s * N + 512:(s + 1) * N])

    nc.compile()
    return nc


_CACHE = {}


def _get_program():
    if "nc" not in _CACHE:
        _CACHE["nc"] = _build_program()
    return _CACHE["nc"]


def _prep_inputs(mask, kernels, scales):
    import ml_dtypes
    bf = ml_dtypes.bfloat16

    atp, ctp99, cc99, ut, U = _host_matrices()

    kers = kernels.astype(np.complex128) * np.sqrt(scales.astype(np.float64))[:, None, None]
    ktR = np.ascontiguousarray(
        kers.real.astype(np.float32).transpose(2, 0, 1).reshape(HK, K * HK))
    ktI = np.ascontiguousarray(
        kers.imag.astype(np.float32).transpose(2, 0, 1).reshape(HK, K * HK))
    # 99-row / 99-col pair-block layout: block p holds kernels (2p, 2p+1) at
    # cols 0:35 / 64:99; rows 0:35 multiply M (kA top), rows 64:99 the swap.
    kri = np.zeros((99, 2 * 12 * 99), np.float32)
    for p in range(12):
        for side, k in ((0, 2 * p), (64, 2 * p + 1)):
            c = p * 99 + side
            kri[0:HK, c:c + HK] = ktR[:, k * HK:(k + 1) * HK]        # t1 top: Kr
            kri[64:99, c:c + HK] = ktI[:, k * HK:(k + 1) * HK]       # t1 bot: Ki
            kri[0:HK, 1188 + c:1188 + c + HK] = ktI[:, k * HK:(k + 1) * HK]
            kri[64:99, 1188 + c:1188 + c + HK] = ktR[:, k * HK:(k + 1) * HK]
    kri = kri.astype(bf)
    # atp: cols 0:560 y-interleaved (atp[8p+r, u], stage 1); cols 560:1352
    # j-chunked 99-col stacks [Ar(35) | zeros(29) | Ai(35)] (stage 1b)
    atp_y = atp.reshape(128, 8 * 2 * HK)                         # [128, 560]
    atp_j = atp.reshape(8, 128, 2 * HK).transpose(1, 0, 2)       # [128, 8, 70]
    atp99 = np.zeros((128, 8, 99), np.float32)
    atp99[:, :, 0:HK] = atp_j[:, :, 0:HK]
    atp99[:, :, 64:99] = atp_j[:, :, HK:2 * HK]
    atp = np.ascontiguousarray(
        np.concatenate([atp_y, atp99.reshape(128, 792)], axis=1))
    cc = np.concatenate([ctp99, cc99], axis=1).astype(bf)      # [99, 288]
    # uht cols permuted so stage-5b's z col 128*(2*tau+s)+p holds output row
    # 256*tau + 2p + s (2 adjacent DRAM rows per partition in the output DMA)
    cidx = np.arange(512)
    rperm = 256 * (cidx // 256) + 2 * (cidx % 128) + ((cidx % 256) // 128)
    uh = [np.ascontiguousarray(U[h * 512:(h + 1) * 512, :].T[:, rperm])
          for h in range(2)]
    uc = [np.concatenate([uh[h], ut], axis=1).astype(bf) for h in range(2)]
    atp_bf = atp.astype(bf)
    mask_bf = np.asarray(mask, np.float32).astype(bf)
    return mask_bf, atp_bf, kri, cc, uc


# ---------------------------------------------------------------- entry point
def kernel(mask, kernels, kernels_ct, scales):
    """Full inputs in, full outputs out.  Shards over 8 NeuronCores internally."""
    from concourse.bass_utils import run_bass_kernel_spmd

    kernels = np.asarray(kernels, np.complex64)
    scales = np.asarray(scales, np.float32)
    mask_bf, atp_bf, kri, cc, uc = _prep_inputs(mask, kernels, scales)

    nc = _get_program()
    in_maps = []
    for c in range(8):
        b, h = c // 2, c % 2
        in_maps.append({
            "x": mask_bf[b],
            "atp": atp_bf,
            "kri": kri,
            "cc": cc,
            "uc": uc[h],
        })

    trace = bool(int(os.environ.get("BASS_KERNEL_TRACE", "0")))
    res = run_bass_kernel_spmd(nc, in_maps, core_ids=list(range(8)), trace=trace)
    _CACHE["last_results"] = res

    aerial = np.empty((B, N, N), np.float32)
    for c in range(8):
        b, h = c // 2, c % 2
        aerial[b, h * 512:(h + 1) * 512, :] = \
            np.asarray(res.results[c]["aerial"]).astype(np.float32)
    resist = (1.0 / (1.0 + np.exp(
        -RESIST_STEEPNESS * (aerial.astype(np.float64) - RESIST_THRESHOLD)
    ))).astype(np.float32)
    printed = (aerial > RESIST_THRESHOLD).astype(np.float32)
    return aerial, resist, printed
